# revision 15
# baseline (speedup 1.0000x reference)
"""Trainium2 Bass kernel for nn_DriftRectifier (2-block Mamba over 64x64 images).

Sharding: data-parallel over batch B=16 -> 2 samples per core x 8 cores.
Inside each core: d_inner=128 lives on SBUF partitions, time (L=4096) on the
free dim.  The selective scan runs as chained [128,512] tensor_tensor_scan
ops; dA comes from ACT exp with per-partition scale A[:,n]; B/C rows are
partition-broadcast via DRAM round-trip DMAs; the n-contraction accumulates
through identity matmuls into PSUM.
"""
import contextlib

import numpy as np

B, C, H, W = 16, 4, 64, 64
L = H * W  # 4096
DM, DI, DS, DK, DR = 64, 128, 16, 4, 4
NCORES = 8
BPC = B // NCORES  # samples per core
TC = 512           # psum / matmul chunk
NCH = L // TC      # 8 chunks
HALF = L // 2      # scan half-sequence
EPS = 1e-5

_CACHE = {}


def _build_program():
    import concourse.bacc as bacc
    import concourse.bass as bass
    from concourse import mybir
    from concourse.tile import TileContext

    F32 = mybir.dt.float32
    BF16 = mybir.dt.bfloat16
    AF = mybir.ActivationFunctionType
    OP = mybir.AluOpType

    nc = bacc.Bacc("TRN2")

    # ---- dram I/O ----
    zc = nc.dram_tensor("zc", [BPC, C, L], F32, kind="ExternalInput")
    out = nc.dram_tensor("out", [BPC, C, L], F32, kind="ExternalOutput")
    ident_in = nc.dram_tensor("ident", [128, 128], BF16, kind="ExternalInput")
    emb_wT = nc.dram_tensor("emb_wT", [C, DM], F32, kind="ExternalInput")
    emb_b = nc.dram_tensor("emb_b", [DM, 1], F32, kind="ExternalInput")
    head_wT = nc.dram_tensor("head_wT", [DM, C], BF16, kind="ExternalInput")
    neg_head_b = nc.dram_tensor("neg_head_b", [C, 1], F32, kind="ExternalInput")
    blk_t = []
    for m in (1, 2):
        p = f"m{m}_"
        blk_t.append({
            "inw_uT": nc.dram_tensor(p + "inw_uT", [DM, DI], BF16, kind="ExternalInput"),
            "inw_zT": nc.dram_tensor(p + "inw_zT", [DM, DI], BF16, kind="ExternalInput"),
            "conv_w": nc.dram_tensor(p + "conv_w", [DI, DK], F32, kind="ExternalInput"),
            "conv_b": nc.dram_tensor(p + "conv_b", [DI, 1], F32, kind="ExternalInput"),
            "xpwT": nc.dram_tensor(p + "xpwT", [DI, DR + 2 * DS], BF16, kind="ExternalInput"),
            "dtpwT": nc.dram_tensor(p + "dtpwT", [DR, DI], BF16, kind="ExternalInput"),
            "dtp_b": nc.dram_tensor(p + "dtp_b", [DI, 1], F32, kind="ExternalInput"),
            "A": nc.dram_tensor(p + "A", [DI, DS], F32, kind="ExternalInput"),
            "D": nc.dram_tensor(p + "D", [DI, 1], F32, kind="ExternalInput"),
            "opwT": nc.dram_tensor(p + "opwT", [DI, DM], BF16, kind="ExternalInput"),
            "ln_g": nc.dram_tensor(p + "ln_g", [DM, 1], F32, kind="ExternalInput"),
            "ln_b": nc.dram_tensor(p + "ln_b", [DM, 1], F32, kind="ExternalInput"),
        })

    with TileContext(nc) as tc, contextlib.ExitStack() as ctx:
        consts = ctx.enter_context(tc.tile_pool(name="consts", bufs=1))
        persist = ctx.enter_context(tc.tile_pool(name="persist", bufs=1))
        nwork = ctx.enter_context(tc.tile_pool(name="nwork", bufs=2))
        bcw = ctx.enter_context(tc.tile_pool(name="bcw", bufs=2))
        small = ctx.enter_context(tc.tile_pool(name="small", bufs=2))
        stats = ctx.enter_context(tc.tile_pool(name="stats", bufs=8))
        stat2 = ctx.enter_context(tc.tile_pool(name="stat2", bufs=1))
        psA = ctx.enter_context(tc.tile_pool(name="psA", bufs=3, space="PSUM"))
        psY = ctx.enter_context(tc.tile_pool(name="psY", bufs=1, space="PSUM"))
        dstage = ctx.enter_context(tc.tile_pool(name="dstage", bufs=2, space="DRAM"))

        # ---- constants to SBUF ----
        ident = consts.tile([128, 128], BF16)
        nc.sync.dma_start(out=ident, in_=ident_in[:])
        sb_embT = consts.tile([C, DM], F32)
        nc.sync.dma_start(out=sb_embT, in_=emb_wT[:])
        sb_embb = consts.tile([DM, 1], F32)
        nc.sync.dma_start(out=sb_embb, in_=emb_b[:])
        sb_headT = consts.tile([DM, C], BF16)
        nc.sync.dma_start(out=sb_headT, in_=head_wT[:])
        sb_nhb = consts.tile([C, 1], F32)
        nc.sync.dma_start(out=sb_nhb, in_=neg_head_b[:])
        ones64 = consts.tile([DM, 1], F32)
        nc.vector.memset(ones64, 1.0)
        ones1x64 = consts.tile([1, DM], F32)
        nc.vector.memset(ones1x64, 1.0)
        eps_t = consts.tile([1, 1], F32)
        nc.vector.memset(eps_t, EPS)
        blk = []
        for m in range(2):
            d = {}
            for k, t in blk_t[m].items():
                d[k] = consts.tile(list(t.shape), t.dtype, name=f"c_m{m}_{k}")
                nc.sync.dma_start(out=d[k], in_=t[:])
            blk.append(d)

        # ---- persistent working tiles (serial across sample-blocks) ----
        zc_sb = persist.tile([C, L], F32)
        feat_bf = persist.tile([DM, L], BF16)
        u_raw = persist.tile([DI, L + DK], BF16)
        u_bf = persist.tile([DI, L], BF16)
        zs_bf = persist.tile([DI, L], BF16)
        dt_f32 = persist.tile([DI, L], F32)
        dtu_bf = persist.tile([DI, L], BF16)
        yo_bf = persist.tile([DI, L], BF16)
        carry = persist.tile([DI, DS], F32)

        for s in range(BPC):
            nc.sync.dma_start(out=zc_sb, in_=zc[s])
            for m in range(2):
                w = blk[m]
                bc_dram = dstage.tile([2 * DS, L], BF16, name="bc_dram")
                with nc.named_scope(f"s{s}m{m}_proj"):
                    if m == 0:
                        for c in range(NCH):
                            cs = slice(c * TC, (c + 1) * TC)
                            ps = psA.tile([DM, TC], F32, name="emb_ps", tag="mm")
                            nc.tensor.matmul(ps, lhsT=sb_embT, rhs=zc_sb[:, cs],
                                             start=True, stop=True)
                            nc.scalar.activation(out=feat_bf[:, cs], in_=ps,
                                                 func=AF.Identity, bias=sb_embb[:, :])
                    nc.vector.memset(u_raw[:, 0:DK - 1], 0.0)
                    # pass 1: silu-set ACT ops only (avoid ACT table thrash)
                    for c in range(NCH):
                        cs = slice(c * TC, (c + 1) * TC)
                        ups = psA.tile([DI, TC], F32, name="ups", tag="mm")
                        nc.tensor.matmul(ups, lhsT=w["inw_uT"], rhs=feat_bf[:, cs],
                                         start=True, stop=True)
                        nc.scalar.activation(
                            out=u_raw[:, DK - 1 + c * TC:DK - 1 + (c + 1) * TC],
                            in_=ups, func=AF.Copy)
                        zps = psA.tile([DI, TC], F32, name="zps", tag="mm")
                        nc.tensor.matmul(zps, lhsT=w["inw_zT"], rhs=feat_bf[:, cs],
                                         start=True, stop=True)
                        nc.scalar.activation(out=zs_bf[:, cs], in_=zps, func=AF.Silu)
                        # depthwise causal conv (K=4) + silu
                        acc = small.tile([DI, TC], F32, name="cacc", tag="cacc")
                        nc.vector.tensor_scalar_mul(acc, u_raw[:, c * TC:c * TC + TC],
                                                    w["conv_w"][:, 0:1])
                        for k in range(1, DK):
                            nc.vector.scalar_tensor_tensor(
                                out=acc, in0=u_raw[:, c * TC + k:c * TC + k + TC],
                                scalar=w["conv_w"][:, k:k + 1], in1=acc,
                                op0=OP.mult, op1=OP.add)
                        nc.scalar.activation(out=u_bf[:, cs], in_=acc, func=AF.Silu,
                                             bias=w["conv_b"][:, :])
                    # pass 2: exp/ln-set ACT ops only
                    for c in range(NCH):
                        cs = slice(c * TC, (c + 1) * TC)
                        xps = psA.tile([DR + 2 * DS, TC], F32, name="xps", tag="mm")
                        nc.tensor.matmul(xps, lhsT=w["xpwT"], rhs=u_bf[:, cs],
                                         start=True, stop=True)
                        # x_proj rows are host-permuted to [B(16), C(16), dt(4)] so
                        # PSUM partition slices start at 0 and 32 (hw: multiples of 32)
                        bcc = small.tile([2 * DS, TC], BF16, name="bcc", tag="bcc")
                        nc.vector.tensor_copy(out=bcc, in_=xps[0:2 * DS, :])
                        nc.sync.dma_start(out=bc_dram[:, cs], in_=bcc)
                        dtr = small.tile([DR, TC], BF16, name="dtr", tag="dtr")
                        nc.vector.tensor_copy(out=dtr, in_=xps[2 * DS:2 * DS + DR, :])
                        dtps = psA.tile([DI, TC], F32, name="dtps", tag="mm")
                        nc.tensor.matmul(dtps, lhsT=w["dtpwT"], rhs=dtr,
                                         start=True, stop=True)
                        # softplus(x) = ln(1 + exp(x)); no ACT table has softplus
                        spe = small.tile([DI, TC], F32, name="spe", tag="spe")
                        nc.scalar.activation(out=spe, in_=dtps, func=AF.Exp,
                                             bias=w["dtp_b"][:, :])
                        nc.vector.tensor_scalar_add(spe, spe, 1.0)
                        nc.scalar.activation(out=dt_f32[:, cs], in_=spe, func=AF.Ln)
                        nc.gpsimd.tensor_tensor(out=dtu_bf[:, cs], in0=dt_f32[:, cs],
                                                in1=u_bf[:, cs], op=OP.mult)

                with nc.named_scope(f"s{s}m{m}_scan"):
                    QSEQ = 1024
                    for quarter in range(L // QSEQ):
                        hs = quarter * QSEQ
                        yps = [psY.tile([DI, TC], F32, name=f"yps{q}", tag=f"yps{q}")
                               for q in range(QSEQ // TC)]
                        for n in range(DS):
                            en = nwork.tile([DI, QSEQ], F32, name="en", tag="en")
                            nc.scalar.activation(out=en, in_=dt_f32[:, hs:hs + QSEQ],
                                                 func=AF.Exp,
                                                 scale=w["A"][:, n:n + 1])
                            bc_t = bcw.tile([DI, QSEQ], BF16, name="bc_t", tag="bc_t")
                            src_b = bass.AP(tensor=bc_dram.tensor,
                                            offset=bc_dram.offset + n * L + hs,
                                            ap=[[0, DI], [1, QSEQ]])
                            nc.sync.dma_start(out=bc_t, in_=src_b)
                            cc_t = bcw.tile([DI, QSEQ], BF16, name="cc_t", tag="cc_t")
                            src_c = bass.AP(tensor=bc_dram.tensor,
                                            offset=bc_dram.offset + (DS + n) * L + hs,
                                            ap=[[0, DI], [1, QSEQ]])
                            nc.sync.dma_start(out=cc_t, in_=src_c)
                            dbu = nwork.tile([DI, QSEQ], BF16, name="dbu", tag="dbu")
                            nc.vector.tensor_tensor(out=dbu, in0=dtu_bf[:, hs:hs + QSEQ],
                                                    in1=bc_t, op=OP.mult)
                            h_t = nwork.tile([DI, QSEQ], BF16, name="h_t", tag="h_t")
                            init = 0.0 if quarter == 0 else carry[:, n:n + 1]
                            nc.vector.tensor_tensor_scan(
                                out=h_t, data0=en, data1=dbu,
                                initial=init, op0=OP.mult, op1=OP.add)
                            if quarter < L // QSEQ - 1:
                                nc.gpsimd.tensor_copy(out=carry[:, n:n + 1],
                                                      in_=h_t[:, QSEQ - 1:QSEQ])
                            hc = nwork.tile([DI, QSEQ], BF16, name="hc", tag="hc")
                            nc.vector.tensor_tensor(out=hc, in0=h_t, in1=cc_t, op=OP.mult)
                            for q in range(QSEQ // TC):
                                nc.tensor.matmul(yps[q], lhsT=ident,
                                                 rhs=hc[:, q * TC:(q + 1) * TC],
                                                 start=(n == 0), stop=(n == DS - 1))
                        for q in range(QSEQ // TC):
                            qs = slice(hs + q * TC, hs + (q + 1) * TC)
                            tmp = small.tile([DI, TC], F32, name="ytmp", tag="ytmp")
                            nc.vector.scalar_tensor_tensor(
                                out=tmp, in0=u_bf[:, qs], scalar=w["D"][:, :],
                                in1=yps[q], op0=OP.mult, op1=OP.add)
                            nc.gpsimd.tensor_tensor(out=yo_bf[:, qs], in0=tmp,
                                                    in1=zs_bf[:, qs], op=OP.mult)

                with nc.named_scope(f"s{s}m{m}_post"):
                    mus, rss, fchs = [], [], []
                    for c in range(NCH):
                        cs = slice(c * TC, (c + 1) * TC)
                        fps = psA.tile([DM, TC], F32, name="fps", tag="mm")
                        nc.tensor.matmul(fps, lhsT=w["opwT"], rhs=yo_bf[:, cs],
                                         start=True, stop=True)
                        fch = stats.tile([DM, TC], F32, name="fch", tag="fch")
                        nc.scalar.activation(out=fch, in_=fps, func=AF.Copy)
                        sq = small.tile([DM, TC], F32, name="sq", tag="sq")
                        nc.scalar.activation(out=sq, in_=fch, func=AF.Square)
                        sps = psA.tile([1, TC], F32, name="sps", tag="mm")
                        nc.tensor.matmul(sps, lhsT=ones64, rhs=fch,
                                         start=True, stop=True)
                        qps = psA.tile([1, TC], F32, name="qps", tag="mm")
                        nc.tensor.matmul(qps, lhsT=ones64, rhs=sq, start=True, stop=True)
                        mu = stats.tile([1, TC], F32, name="mu", tag="mu")
                        nc.vector.tensor_scalar_mul(mu, sps, 1.0 / DM)
                        msq = stat2.tile([1, TC], F32, name="msq", tag="msq")
                        nc.vector.tensor_scalar_mul(msq, qps, 1.0 / DM)
                        mu2 = stat2.tile([1, TC], F32, name="mu2", tag="mu2")
                        nc.vector.tensor_tensor(out=mu2, in0=mu, in1=mu, op=OP.mult)
                        var = stat2.tile([1, TC], F32, name="var", tag="var")
                        nc.vector.tensor_tensor(out=var, in0=msq, in1=mu2, op=OP.subtract)
                        # rstd = exp(-0.5*ln(var+eps)); keeps ACT in the exp/ln set
                        lnv = stat2.tile([1, TC], F32, name="lnv", tag="lnv")
                        nc.scalar.activation(out=lnv, in_=var, func=AF.Ln, bias=eps_t[:, :])
                        rs = stats.tile([1, TC], F32, name="rs", tag="rs")
                        nc.scalar.activation(out=rs, in_=lnv, func=AF.Exp, scale=-0.5)
                        mus.append(mu)
                        rss.append(rs)
                        fchs.append(fch)
                    for c in range(NCH):
                        cs = slice(c * TC, (c + 1) * TC)
                        mups = psA.tile([DM, TC], F32, name="mups", tag="mm")
                        nc.tensor.matmul(mups, lhsT=ones1x64, rhs=mus[c], start=True, stop=True)
                        rsps = psA.tile([DM, TC], F32, name="rsps", tag="mm")
                        nc.tensor.matmul(rsps, lhsT=ones1x64, rhs=rss[c], start=True, stop=True)
                        t1 = small.tile([DM, TC], F32, name="t1", tag="t1")
                        nc.vector.tensor_tensor(out=t1, in0=fchs[c], in1=mups,
                                                op=OP.subtract)
                        t2 = small.tile([DM, TC], F32, name="t2", tag="t2")
                        nc.vector.tensor_tensor(out=t2, in0=t1, in1=rsps, op=OP.mult)
                        nc.scalar.activation(out=feat_bf[:, cs], in_=t2, func=AF.Identity,
                                             scale=w["ln_g"][:, :], bias=w["ln_b"][:, :])
                        if m == 1:
                            dps = psA.tile([C, TC], F32, name="dps", tag="mm")
                            nc.tensor.matmul(dps, lhsT=sb_headT, rhs=feat_bf[:, cs],
                                             start=True, stop=True)
                            nd = small.tile([C, TC], F32, name="nd", tag="nd")
                            nc.scalar.activation(out=nd, in_=dps, func=AF.Identity,
                                                 scale=-1.0, bias=sb_nhb[:, :])
                            oc = small.tile([C, TC], F32, name="oc", tag="oc")
                            nc.vector.tensor_tensor(out=oc, in0=zc_sb[:, cs], in1=nd,
                                                    op=OP.add)
                            nc.sync.dma_start(out=out[s][:, cs], in_=oc)

    nc.finalize()
    return nc


def _prep_maps(inputs):
    import ml_dtypes
    bf = ml_dtypes.bfloat16
    f = np.float32
    z = np.asarray(inputs["z_damaged"], dtype=f).reshape(B, C, L)

    base = {
        "ident": np.eye(128, dtype=bf),
        "emb_wT": np.ascontiguousarray(np.asarray(inputs["emb_w"], f).T),
        "emb_b": np.asarray(inputs["emb_b"], f).reshape(DM, 1),
        "head_wT": np.ascontiguousarray(np.asarray(inputs["head_w"], f).T).astype(bf),
        "neg_head_b": (-np.asarray(inputs["head_b"], f)).reshape(C, 1),
    }
    for m in (1, 2):
        p = f"m{m}_"
        inw = np.asarray(inputs[p + "in_proj_w"], f)  # [2DI, DM]
        base[p + "inw_uT"] = np.ascontiguousarray(inw[:DI].T).astype(bf)
        base[p + "inw_zT"] = np.ascontiguousarray(inw[DI:].T).astype(bf)
        base[p + "conv_w"] = np.ascontiguousarray(
            np.asarray(inputs[p + "conv_w"], f).reshape(DI, DK))
        base[p + "conv_b"] = np.asarray(inputs[p + "conv_b"], f).reshape(DI, 1)
        xpw = np.asarray(inputs[p + "x_proj_w"], f)  # rows: dt(4), B(16), C(16)
        xpw = np.concatenate([xpw[DR:], xpw[:DR]], axis=0)  # -> B, C, dt
        base[p + "xpwT"] = np.ascontiguousarray(xpw.T).astype(bf)
        base[p + "dtpwT"] = np.ascontiguousarray(
            np.asarray(inputs[p + "dt_proj_w"], f).T).astype(bf)
        base[p + "dtp_b"] = np.asarray(inputs[p + "dt_proj_b"], f).reshape(DI, 1)
        base[p + "A"] = -np.exp(np.asarray(inputs[p + "A_log"], f))
        base[p + "D"] = np.asarray(inputs[p + "D"], f).reshape(DI, 1)
        base[p + "opwT"] = np.ascontiguousarray(
            np.asarray(inputs[p + "out_proj_w"], f).T).astype(bf)
        base[p + "ln_g"] = np.asarray(inputs[f"ln{m}_g"], f).reshape(DM, 1)
        base[p + "ln_b"] = np.asarray(inputs[f"ln{m}_b"], f).reshape(DM, 1)

    maps = []
    for k in range(NCORES):
        mkp = dict(base)
        mkp["zc"] = np.ascontiguousarray(z[k * BPC:(k + 1) * BPC])
        maps.append(mkp)
    return maps


def _run(inputs, trace=False):
    from concourse.bass_utils import run_bass_kernel_spmd
    if "nc" not in _CACHE:
        _CACHE["nc"] = _build_program()
    nc = _CACHE["nc"]
    maps = _prep_maps(inputs)
    res = run_bass_kernel_spmd(nc, maps, core_ids=list(range(NCORES)), trace=trace)
    outs = [r["out"] for r in res.results]
    full = np.concatenate(outs, axis=0).reshape(B, C, H, W)
    return full, res


def kernel(**inputs):
    full, _ = _run(inputs, trace=False)
    return full


# revision 16
# speedup vs baseline: 1.1951x; 1.1951x over previous
"""Trainium2 Bass kernel for nn_DriftRectifier (2-block Mamba over 64x64 images).

Sharding: data-parallel over batch B=16 -> 2 samples per core x 8 cores.
Inside each core: d_inner=128 lives on SBUF partitions, time (L=4096) on the
free dim.  The selective scan runs as chained [128,512] tensor_tensor_scan
ops; dA comes from ACT exp with per-partition scale A[:,n]; B/C rows are
partition-broadcast via DRAM round-trip DMAs; the n-contraction accumulates
through identity matmuls into PSUM.
"""
import contextlib

import numpy as np

B, C, H, W = 16, 4, 64, 64
L = H * W  # 4096
DM, DI, DS, DK, DR = 64, 128, 16, 4, 4
NCORES = 8
BPC = B // NCORES  # samples per core
TC = 512           # psum / matmul chunk
NCH = L // TC      # 8 chunks
HALF = L // 2      # scan half-sequence
EPS = 1e-5

_CACHE = {}


def _build_program():
    import concourse.bacc as bacc
    import concourse.bass as bass
    from concourse import mybir
    from concourse.tile import TileContext

    F32 = mybir.dt.float32
    BF16 = mybir.dt.bfloat16
    AF = mybir.ActivationFunctionType
    OP = mybir.AluOpType

    nc = bacc.Bacc("TRN2")

    # ---- dram I/O ----
    zc = nc.dram_tensor("zc", [BPC, C, L], F32, kind="ExternalInput")
    out = nc.dram_tensor("out", [BPC, C, L], F32, kind="ExternalOutput")
    ident_in = nc.dram_tensor("ident", [128, 128], BF16, kind="ExternalInput")
    emb_wT = nc.dram_tensor("emb_wT", [C, DM], F32, kind="ExternalInput")
    emb_b = nc.dram_tensor("emb_b", [DM, 1], F32, kind="ExternalInput")
    head_wT = nc.dram_tensor("head_wT", [DM, C], BF16, kind="ExternalInput")
    neg_head_b = nc.dram_tensor("neg_head_b", [C, 1], F32, kind="ExternalInput")
    blk_t = []
    for m in (1, 2):
        p = f"m{m}_"
        blk_t.append({
            "inw_uT": nc.dram_tensor(p + "inw_uT", [DM, DI], BF16, kind="ExternalInput"),
            "inw_zT": nc.dram_tensor(p + "inw_zT", [DM, DI], BF16, kind="ExternalInput"),
            "conv_w": nc.dram_tensor(p + "conv_w", [DI, DK], F32, kind="ExternalInput"),
            "conv_b": nc.dram_tensor(p + "conv_b", [DI, 1], F32, kind="ExternalInput"),
            "xpwT": nc.dram_tensor(p + "xpwT", [DI, DR + 2 * DS], BF16, kind="ExternalInput"),
            "dtpwT": nc.dram_tensor(p + "dtpwT", [DR, DI], BF16, kind="ExternalInput"),
            "dtp_b": nc.dram_tensor(p + "dtp_b", [DI, 1], F32, kind="ExternalInput"),
            "A": nc.dram_tensor(p + "A", [DI, DS], F32, kind="ExternalInput"),
            "D": nc.dram_tensor(p + "D", [DI, 1], F32, kind="ExternalInput"),
            "opwT": nc.dram_tensor(p + "opwT", [DI, DM], BF16, kind="ExternalInput"),
            "ln_g": nc.dram_tensor(p + "ln_g", [DM, 1], F32, kind="ExternalInput"),
            "ln_b": nc.dram_tensor(p + "ln_b", [DM, 1], F32, kind="ExternalInput"),
        })

    with TileContext(nc) as tc, contextlib.ExitStack() as ctx:
        consts = ctx.enter_context(tc.tile_pool(name="consts", bufs=1))
        persist = ctx.enter_context(tc.tile_pool(name="persist", bufs=1))
        nwork = ctx.enter_context(tc.tile_pool(name="nwork", bufs=2))
        bcw = ctx.enter_context(tc.tile_pool(name="bcw", bufs=4))
        small = ctx.enter_context(tc.tile_pool(name="small", bufs=2))
        stats = ctx.enter_context(tc.tile_pool(name="stats", bufs=8))
        stat2 = ctx.enter_context(tc.tile_pool(name="stat2", bufs=1))
        psA = ctx.enter_context(tc.tile_pool(name="psA", bufs=3, space="PSUM"))
        psY = ctx.enter_context(tc.tile_pool(name="psY", bufs=1, space="PSUM"))
        dstage = ctx.enter_context(tc.tile_pool(name="dstage", bufs=2, space="DRAM"))

        # ---- constants to SBUF ----
        ident = consts.tile([128, 128], BF16)
        nc.sync.dma_start(out=ident, in_=ident_in[:])
        sb_embT = consts.tile([C, DM], F32)
        nc.sync.dma_start(out=sb_embT, in_=emb_wT[:])
        sb_embb = consts.tile([DM, 1], F32)
        nc.sync.dma_start(out=sb_embb, in_=emb_b[:])
        sb_headT = consts.tile([DM, C], BF16)
        nc.sync.dma_start(out=sb_headT, in_=head_wT[:])
        sb_nhb = consts.tile([C, 1], F32)
        nc.sync.dma_start(out=sb_nhb, in_=neg_head_b[:])
        ones64 = consts.tile([DM, 1], F32)
        nc.vector.memset(ones64, 1.0)
        ones1x64 = consts.tile([1, DM], F32)
        nc.vector.memset(ones1x64, 1.0)
        eps_t = consts.tile([1, 1], F32)
        nc.vector.memset(eps_t, EPS)
        blk = []
        for m in range(2):
            d = {}
            for k, t in blk_t[m].items():
                d[k] = consts.tile(list(t.shape), t.dtype, name=f"c_m{m}_{k}")
                nc.sync.dma_start(out=d[k], in_=t[:])
            blk.append(d)

        # ---- persistent working tiles (serial across sample-blocks) ----
        zc_sb = persist.tile([C, L], F32)
        feat_bf = persist.tile([DM, L], BF16)
        u_raw = persist.tile([DI, L + DK], BF16)
        u_bf = persist.tile([DI, L], BF16)
        zs_bf = persist.tile([DI, L], BF16)
        dt_f32 = persist.tile([DI, L], F32)
        dtu_bf = persist.tile([DI, L], BF16)
        yo_bf = persist.tile([DI, L], BF16)
        carry = persist.tile([DI, DS], F32)

        for s in range(BPC):
            nc.sync.dma_start(out=zc_sb, in_=zc[s])
            for m in range(2):
                w = blk[m]
                bc_dram = dstage.tile([2 * DS, L], BF16, name="bc_dram")
                with nc.named_scope(f"s{s}m{m}_proj"):
                    if m == 0:
                        for c in range(NCH):
                            cs = slice(c * TC, (c + 1) * TC)
                            ps = psA.tile([DM, TC], F32, name="emb_ps", tag="mm")
                            nc.tensor.matmul(ps, lhsT=sb_embT, rhs=zc_sb[:, cs],
                                             start=True, stop=True)
                            nc.scalar.activation(out=feat_bf[:, cs], in_=ps,
                                                 func=AF.Identity, bias=sb_embb[:, :])
                    nc.vector.memset(u_raw[:, 0:DK - 1], 0.0)
                    # pass 1: silu-set ACT ops only (avoid ACT table thrash)
                    for c in range(NCH):
                        cs = slice(c * TC, (c + 1) * TC)
                        ups = psA.tile([DI, TC], F32, name="ups", tag="mm")
                        nc.tensor.matmul(ups, lhsT=w["inw_uT"], rhs=feat_bf[:, cs],
                                         start=True, stop=True)
                        nc.scalar.activation(
                            out=u_raw[:, DK - 1 + c * TC:DK - 1 + (c + 1) * TC],
                            in_=ups, func=AF.Copy)
                        zps = psA.tile([DI, TC], F32, name="zps", tag="mm")
                        nc.tensor.matmul(zps, lhsT=w["inw_zT"], rhs=feat_bf[:, cs],
                                         start=True, stop=True)
                        nc.scalar.activation(out=zs_bf[:, cs], in_=zps, func=AF.Silu)
                        # depthwise causal conv (K=4) + silu
                        acc = small.tile([DI, TC], F32, name="cacc", tag="cacc")
                        nc.vector.tensor_scalar_mul(acc, u_raw[:, c * TC:c * TC + TC],
                                                    w["conv_w"][:, 0:1])
                        for k in range(1, DK):
                            nc.vector.scalar_tensor_tensor(
                                out=acc, in0=u_raw[:, c * TC + k:c * TC + k + TC],
                                scalar=w["conv_w"][:, k:k + 1], in1=acc,
                                op0=OP.mult, op1=OP.add)
                        nc.scalar.activation(out=u_bf[:, cs], in_=acc, func=AF.Silu,
                                             bias=w["conv_b"][:, :])
                    # pass 2: exp/ln-set ACT ops only
                    for c in range(NCH):
                        cs = slice(c * TC, (c + 1) * TC)
                        xps = psA.tile([DR + 2 * DS, TC], F32, name="xps", tag="mm")
                        nc.tensor.matmul(xps, lhsT=w["xpwT"], rhs=u_bf[:, cs],
                                         start=True, stop=True)
                        # x_proj rows are host-permuted to [B(16), C(16), dt(4)] so
                        # PSUM partition slices start at 0 and 32 (hw: multiples of 32)
                        bcc = small.tile([2 * DS, TC], BF16, name="bcc", tag="bcc")
                        nc.vector.tensor_copy(out=bcc, in_=xps[0:2 * DS, :])
                        nc.sync.dma_start(out=bc_dram[:, cs], in_=bcc)
                        dtr = small.tile([DR, TC], BF16, name="dtr", tag="dtr")
                        nc.vector.tensor_copy(out=dtr, in_=xps[2 * DS:2 * DS + DR, :])
                        dtps = psA.tile([DI, TC], F32, name="dtps", tag="mm")
                        nc.tensor.matmul(dtps, lhsT=w["dtpwT"], rhs=dtr,
                                         start=True, stop=True)
                        # softplus(x) = ln(1 + exp(x)); no ACT table has softplus
                        spe = small.tile([DI, TC], F32, name="spe", tag="cacc")
                        nc.scalar.activation(out=spe, in_=dtps, func=AF.Exp,
                                             bias=w["dtp_b"][:, :])
                        nc.vector.tensor_scalar_add(spe, spe, 1.0)
                        nc.scalar.activation(out=dt_f32[:, cs], in_=spe, func=AF.Ln)
                        nc.gpsimd.tensor_tensor(out=dtu_bf[:, cs], in0=dt_f32[:, cs],
                                                in1=u_bf[:, cs], op=OP.mult)

                with nc.named_scope(f"s{s}m{m}_scan"):
                    QSEQ = 1024
                    for quarter in range(L // QSEQ):
                        hs = quarter * QSEQ
                        yps = [psY.tile([DI, TC], F32, name=f"yps{q}", tag=f"yps{q}")
                               for q in range(QSEQ // TC)]
                        for n in range(DS):
                            en = nwork.tile([DI, QSEQ], F32, name="en", tag="en")
                            nc.scalar.activation(out=en, in_=dt_f32[:, hs:hs + QSEQ],
                                                 func=AF.Exp,
                                                 scale=w["A"][:, n:n + 1])
                            bc_t = bcw.tile([DI, QSEQ], BF16, name="bc_t", tag="bc_t")
                            src_b = bass.AP(tensor=bc_dram.tensor,
                                            offset=bc_dram.offset + n * L + hs,
                                            ap=[[0, DI], [1, QSEQ]])
                            nc.sync.dma_start(out=bc_t, in_=src_b)
                            cc_t = bcw.tile([DI, QSEQ], BF16, name="cc_t", tag="cc_t")
                            src_c = bass.AP(tensor=bc_dram.tensor,
                                            offset=bc_dram.offset + (DS + n) * L + hs,
                                            ap=[[0, DI], [1, QSEQ]])
                            nc.gpsimd.dma_start(out=cc_t, in_=src_c)
                            dbu = nwork.tile([DI, QSEQ], BF16, name="dbu", tag="dbu")
                            nc.vector.tensor_tensor(out=dbu, in0=dtu_bf[:, hs:hs + QSEQ],
                                                    in1=bc_t, op=OP.mult)
                            h_t = nwork.tile([DI, QSEQ], BF16, name="h_t", tag="h_t")
                            init = 0.0 if quarter == 0 else carry[:, n:n + 1]
                            nc.vector.tensor_tensor_scan(
                                out=h_t, data0=en, data1=dbu,
                                initial=init, op0=OP.mult, op1=OP.add)
                            if quarter < L // QSEQ - 1:
                                nc.vector.tensor_copy(out=carry[:, n:n + 1],
                                                      in_=h_t[:, QSEQ - 1:QSEQ])
                            hc = nwork.tile([DI, QSEQ], BF16, name="hc", tag="hc")
                            nc.vector.tensor_tensor(out=hc, in0=h_t, in1=cc_t, op=OP.mult)
                            for q in range(QSEQ // TC):
                                nc.tensor.matmul(yps[q], lhsT=ident,
                                                 rhs=hc[:, q * TC:(q + 1) * TC],
                                                 start=(n == 0), stop=(n == DS - 1))
                        for q in range(QSEQ // TC):
                            qs = slice(hs + q * TC, hs + (q + 1) * TC)
                            tmp = small.tile([DI, TC], F32, name="ytmp", tag="ytmp")
                            nc.vector.scalar_tensor_tensor(
                                out=tmp, in0=u_bf[:, qs], scalar=w["D"][:, :],
                                in1=yps[q], op0=OP.mult, op1=OP.add)
                            nc.gpsimd.tensor_tensor(out=yo_bf[:, qs], in0=tmp,
                                                    in1=zs_bf[:, qs], op=OP.mult)

                with nc.named_scope(f"s{s}m{m}_post"):
                    mus, rss, fchs = [], [], []
                    for c in range(NCH):
                        cs = slice(c * TC, (c + 1) * TC)
                        fps = psA.tile([DM, TC], F32, name="fps", tag="mm")
                        nc.tensor.matmul(fps, lhsT=w["opwT"], rhs=yo_bf[:, cs],
                                         start=True, stop=True)
                        fch = stats.tile([DM, TC], F32, name="fch", tag="fch")
                        nc.scalar.activation(out=fch, in_=fps, func=AF.Copy)
                        sq = small.tile([DM, TC], F32, name="sq", tag="sq")
                        nc.scalar.activation(out=sq, in_=fch, func=AF.Square)
                        sps = psA.tile([1, TC], F32, name="sps", tag="mm")
                        nc.tensor.matmul(sps, lhsT=ones64, rhs=fch,
                                         start=True, stop=True)
                        qps = psA.tile([1, TC], F32, name="qps", tag="mm")
                        nc.tensor.matmul(qps, lhsT=ones64, rhs=sq, start=True, stop=True)
                        mu = stats.tile([1, TC], F32, name="mu", tag="mu")
                        nc.vector.tensor_scalar_mul(mu, sps, 1.0 / DM)
                        msq = stat2.tile([1, TC], F32, name="msq", tag="msq")
                        nc.vector.tensor_scalar_mul(msq, qps, 1.0 / DM)
                        mu2 = stat2.tile([1, TC], F32, name="mu2", tag="mu2")
                        nc.vector.tensor_tensor(out=mu2, in0=mu, in1=mu, op=OP.mult)
                        var = stat2.tile([1, TC], F32, name="var", tag="var")
                        nc.vector.tensor_tensor(out=var, in0=msq, in1=mu2, op=OP.subtract)
                        # rstd = exp(-0.5*ln(var+eps)); keeps ACT in the exp/ln set
                        lnv = stat2.tile([1, TC], F32, name="lnv", tag="lnv")
                        nc.scalar.activation(out=lnv, in_=var, func=AF.Ln, bias=eps_t[:, :])
                        rs = stats.tile([1, TC], F32, name="rs", tag="rs")
                        nc.scalar.activation(out=rs, in_=lnv, func=AF.Exp, scale=-0.5)
                        mus.append(mu)
                        rss.append(rs)
                        fchs.append(fch)
                    for c in range(NCH):
                        cs = slice(c * TC, (c + 1) * TC)
                        mups = psA.tile([DM, TC], F32, name="mups", tag="mm")
                        nc.tensor.matmul(mups, lhsT=ones1x64, rhs=mus[c], start=True, stop=True)
                        rsps = psA.tile([DM, TC], F32, name="rsps", tag="mm")
                        nc.tensor.matmul(rsps, lhsT=ones1x64, rhs=rss[c], start=True, stop=True)
                        t1 = small.tile([DM, TC], F32, name="t1", tag="t1")
                        nc.vector.tensor_tensor(out=t1, in0=fchs[c], in1=mups,
                                                op=OP.subtract)
                        t2 = small.tile([DM, TC], F32, name="t2", tag="t2")
                        nc.vector.tensor_tensor(out=t2, in0=t1, in1=rsps, op=OP.mult)
                        nc.scalar.activation(out=feat_bf[:, cs], in_=t2, func=AF.Identity,
                                             scale=w["ln_g"][:, :], bias=w["ln_b"][:, :])
                        if m == 1:
                            dps = psA.tile([C, TC], F32, name="dps", tag="mm")
                            nc.tensor.matmul(dps, lhsT=sb_headT, rhs=feat_bf[:, cs],
                                             start=True, stop=True)
                            nd = small.tile([C, TC], F32, name="nd", tag="oc")
                            nc.scalar.activation(out=nd, in_=dps, func=AF.Identity,
                                                 scale=-1.0, bias=sb_nhb[:, :])
                            oc = small.tile([C, TC], F32, name="oc", tag="oc")
                            nc.vector.tensor_tensor(out=oc, in0=zc_sb[:, cs], in1=nd,
                                                    op=OP.add)
                            nc.sync.dma_start(out=out[s][:, cs], in_=oc)

    nc.finalize()
    return nc


def _prep_maps(inputs):
    import ml_dtypes
    bf = ml_dtypes.bfloat16
    f = np.float32
    z = np.asarray(inputs["z_damaged"], dtype=f).reshape(B, C, L)

    base = {
        "ident": np.eye(128, dtype=bf),
        "emb_wT": np.ascontiguousarray(np.asarray(inputs["emb_w"], f).T),
        "emb_b": np.asarray(inputs["emb_b"], f).reshape(DM, 1),
        "head_wT": np.ascontiguousarray(np.asarray(inputs["head_w"], f).T).astype(bf),
        "neg_head_b": (-np.asarray(inputs["head_b"], f)).reshape(C, 1),
    }
    for m in (1, 2):
        p = f"m{m}_"
        inw = np.asarray(inputs[p + "in_proj_w"], f)  # [2DI, DM]
        base[p + "inw_uT"] = np.ascontiguousarray(inw[:DI].T).astype(bf)
        base[p + "inw_zT"] = np.ascontiguousarray(inw[DI:].T).astype(bf)
        base[p + "conv_w"] = np.ascontiguousarray(
            np.asarray(inputs[p + "conv_w"], f).reshape(DI, DK))
        base[p + "conv_b"] = np.asarray(inputs[p + "conv_b"], f).reshape(DI, 1)
        xpw = np.asarray(inputs[p + "x_proj_w"], f)  # rows: dt(4), B(16), C(16)
        xpw = np.concatenate([xpw[DR:], xpw[:DR]], axis=0)  # -> B, C, dt
        base[p + "xpwT"] = np.ascontiguousarray(xpw.T).astype(bf)
        base[p + "dtpwT"] = np.ascontiguousarray(
            np.asarray(inputs[p + "dt_proj_w"], f).T).astype(bf)
        base[p + "dtp_b"] = np.asarray(inputs[p + "dt_proj_b"], f).reshape(DI, 1)
        base[p + "A"] = -np.exp(np.asarray(inputs[p + "A_log"], f))
        base[p + "D"] = np.asarray(inputs[p + "D"], f).reshape(DI, 1)
        base[p + "opwT"] = np.ascontiguousarray(
            np.asarray(inputs[p + "out_proj_w"], f).T).astype(bf)
        base[p + "ln_g"] = np.asarray(inputs[f"ln{m}_g"], f).reshape(DM, 1)
        base[p + "ln_b"] = np.asarray(inputs[f"ln{m}_b"], f).reshape(DM, 1)

    maps = []
    for k in range(NCORES):
        mkp = dict(base)
        mkp["zc"] = np.ascontiguousarray(z[k * BPC:(k + 1) * BPC])
        maps.append(mkp)
    return maps


def _run(inputs, trace=False):
    from concourse.bass_utils import run_bass_kernel_spmd
    if "nc" not in _CACHE:
        _CACHE["nc"] = _build_program()
    nc = _CACHE["nc"]
    maps = _prep_maps(inputs)
    res = run_bass_kernel_spmd(nc, maps, core_ids=list(range(NCORES)), trace=trace)
    outs = [r["out"] for r in res.results]
    full = np.concatenate(outs, axis=0).reshape(B, C, H, W)
    return full, res


def kernel(**inputs):
    full, _ = _run(inputs, trace=False)
    return full
